# revision 4
# baseline (speedup 1.0000x reference)
"""Trainium2 Bass kernel for nn_ModelNew_17411797418162.

Computation (per (b,s) sample):
  mixed = h_res @ x            # [4,4] @ [4,1024]
  out   = mixed * h_out[None,:] + h_post[:,None] * x

Sharding: pure data parallel over the leading batch dim B=8 -> 1 batch/core.

Per-core design (memory-bound). All device I/O is fp16 (host-side cast;
rel err ~1.5e-3 vs fp32 reference, well under the 2e-2 gate), which halves
HBM traffic vs the fp32 baseline: x 16MB + out 16MB + h_out 4MB +
weights 4MB ~= 40MB -> ~135-145us DMA roofline/core.

- Flatten (s, stream) -> rows: x/out [8192, 1024] fp16; loads and stores
  are contiguous 2KB-per-partition-line DMAs.
- Rewrite out = (h_res @ (x * h_out_bcast)) + diag(h_post) @ x:
  * h_out is broadcast onto the 4 stream rows by a constant 0/1 matmul on
    the PE (e4), landing in PSUM.
  * DVE does the single elementwise op: y = x * h_out_bcast (fp16 out).
  * The per-sample 4x4 GEMM runs as exact block-diagonal [128,128] fp16
    matmuls covering 32 samples each, and the h_post residual term is a
    *diagonal* [128,128] fp16 matmul accumulated into the same PSUM group
    (start/stop flags), so no separate vector add is needed.
  * ACT evacuates the final PSUM to fp16 SBUF; one store DMA per 512 rows.
- Per-[128,1024] block engine cost: DVE ~1.2us, ACT ~0.95us, PE ~1.3us
  (6 fp16 matmuls), all under the ~2.2us DMA slot -> DMA-bound.
"""
import numpy as np

import concourse.bacc as bacc
import concourse.tile as tile
import concourse.mybir as mybir
from concourse.bass_utils import run_bass_kernel_spmd

B, S, N, D = 8, 2048, 4, 1024
NCORES = 8
ROWS = S * N              # 8192 flattened rows per core
NSB = 16                  # super-blocks per core (512 rows each)
SUBS = 4                  # sub-blocks (32 samples x 4 streams) per super-block
F32 = mybir.dt.float32
FP16 = mybir.dt.float16

_cache = {}


def build_program(iters: int = 1, mode: str = "full"):
    """Build the SPMD Bass program (one core's view). Cached per (iters, mode).

    mode: "full" = real kernel; ablations for bottleneck isolation:
      "dma"   = loads + store only (wrong output values)
      "nodve" = full minus the DVE multiply (wrong values)
      "nope"  = loads, DVE mult on x, ACT evac of x, store (no matmuls)
    """
    if (iters, mode) in _cache:
        return _cache[(iters, mode)]

    nc = bacc.Bacc("TRN2", target_bir_lowering=False, debug=False)
    x = nc.dram_tensor("x", [ROWS, D], FP16, kind="ExternalInput")
    w = nc.dram_tensor("w", [128, 64 * 128], FP16, kind="ExternalInput")
    dg = nc.dram_tensor("dg", [128, 64 * 128], FP16, kind="ExternalInput")
    ho = nc.dram_tensor("ho", [S, D], FP16, kind="ExternalInput")
    e4 = nc.dram_tensor("e4", [128, 128], FP16, kind="ExternalInput")
    out = nc.dram_tensor("out", [ROWS, D], FP16, kind="ExternalOutput")

    with tile.TileContext(nc) as tc:
        with (
            tc.tile_pool(name="const", bufs=1) as cpool,
            tc.tile_pool(name="big", bufs=3) as bpool,
            tc.tile_pool(name="mid", bufs=4) as mpool,
            tc.tile_pool(name="psum", bufs=4, space="PSUM") as ppool,
        ):
            e4_t = cpool.tile([128, 128], FP16)
            w_all = cpool.tile([128, 64 * 128], FP16)
            d_all = cpool.tile([128, 64 * 128], FP16)

            def body():
                # Weight loads inside the body so the For_i slope measurement
                # reflects the full per-pass HBM traffic (they are per-pass
                # inputs, not reusable weights).
                nc.gpsimd.dma_start(e4_t[:], e4.ap())
                nc.gpsimd.dma_start(w_all[:], w.ap())
                nc.gpsimd.dma_start(d_all[:], dg.ap())
                for sb in range(NSB):
                    # x rows 512*sb .. 512*(sb+1), tiled [p=128, (k=4, d=1024)]
                    x_t = bpool.tile([128, SUBS * D], FP16, tag="x")
                    src = x.ap()[512 * sb:512 * (sb + 1), :].rearrange(
                        "(k p) d -> p k d", k=SUBS)
                    nc.sync.dma_start(
                        x_t[:].rearrange("p (k d) -> p k d", k=SUBS), src)

                    # h_out fp16 for these 128 samples (partition = sample)
                    ho_t = mpool.tile([128, D], FP16, tag="ho")
                    nc.gpsimd.dma_start(ho_t[:], ho.ap()[128 * sb:128 * (sb + 1)])

                    out_sb = bpool.tile([128, SUBS * D], FP16, tag="out_sb")

                    for k in range(SUBS):
                        if mode == "dma":
                            continue
                        blk = SUBS * sb + k
                        xk = x_t[:, D * k:D * (k + 1)]

                        if mode in ("full", "nodve"):
                            # broadcast h_out onto stream rows via 0/1 matmul
                            psh = ppool.tile([128, D], F32, tag="ps")
                            for c in range(2):
                                nc.tensor.matmul(
                                    psh[:, 512 * c:512 * (c + 1)],
                                    e4_t[32 * k:32 * (k + 1), :],
                                    ho_t[32 * k:32 * (k + 1),
                                         512 * c:512 * (c + 1)],
                                    start=True, stop=True,
                                    tile_position=(32 * k, 0))

                        y_t = mpool.tile([128, D], FP16, tag="y")
                        if mode == "full":
                            nc.vector.tensor_mul(y_t[:], xk, psh)
                        elif mode == "nope":
                            nc.vector.tensor_mul(y_t[:], xk, xk)

                        if mode in ("full", "nodve"):
                            # "nodve": feed raw x to the mix matmul instead of y
                            rhs = y_t[:] if mode == "full" else xk
                            ps = ppool.tile([128, D], F32, tag="ps")
                            lhsW = w_all[:, 128 * blk:128 * (blk + 1)]
                            lhsD = d_all[:, 128 * blk:128 * (blk + 1)]
                            for c in range(2):
                                nc.tensor.matmul(
                                    ps[:, 512 * c:512 * (c + 1)],
                                    lhsW,
                                    rhs[:, 512 * c:512 * (c + 1)],
                                    start=True, stop=False)
                                nc.tensor.matmul(
                                    ps[:, 512 * c:512 * (c + 1)],
                                    lhsD,
                                    x_t[:, D * k + 512 * c:
                                        D * k + 512 * (c + 1)],
                                    start=False, stop=True)
                            nc.scalar.copy(out_sb[:, D * k:D * (k + 1)], ps[:])
                        else:
                            nc.scalar.copy(out_sb[:, D * k:D * (k + 1)], y_t[:])

                    src_sb = x_t if mode == "dma" else out_sb
                    dst = out.ap()[512 * sb:512 * (sb + 1), :].rearrange(
                        "(k p) d -> p k d", k=SUBS)
                    nc.scalar.dma_start(
                        dst, src_sb[:].rearrange("p (k d) -> p k d", k=SUBS))

            if iters == 1:
                body()
            else:
                with tc.For_i(0, iters, 1):
                    body()

    nc.compile()
    _cache[(iters, mode)] = nc
    return nc


def _f16(a):
    """fp16 cast with subnormal flush (HW engines may flush; make it exact)."""
    h = np.asarray(a, np.float32).astype(np.float16)
    h[np.abs(h.astype(np.float32)) < 2.0 ** -14] = 0
    return h


def make_in_maps(x, h_res, h_out, h_post):
    """Split full inputs into per-core input maps (host-side, layout only)."""
    x = np.ascontiguousarray(x, dtype=np.float32)
    h_res = np.ascontiguousarray(h_res, dtype=np.float32)
    h_out = np.ascontiguousarray(h_out, dtype=np.float32)
    h_post = np.ascontiguousarray(h_post, dtype=np.float32)

    # stream-replication matrix: e4[q, 4*(q%32)+i] = 1  (lhsT of the bcast
    # matmul; K-rows live at partitions 32k..32k+32 via tile_position)
    e4 = np.zeros((128, 128), np.float16)
    q = np.arange(128)
    for i in range(4):
        e4[q, 4 * (q % 32) + i] = 1.0

    p32 = np.arange(32)
    r128 = np.arange(128)

    in_maps = []
    for c in range(NCORES):
        xc = _f16(x[c].reshape(ROWS, D))
        # Block-diagonal mixing weights, laid out [r, (b, col)] so the DMA is
        # contiguous 16KB per partition: W[4p+j, b, 4p+i] = h_res[c, 32b+p, i, j]
        hr = _f16(h_res[c].reshape(64, 32, 4, 4))       # [b, p, i, j]
        W = np.zeros((128, 64, 128), np.float16)
        for i in range(4):
            for j in range(4):
                W[4 * p32 + j, :, 4 * p32 + i] = hr[:, p32, i, j].T
        # Diagonal h_post weights: Dg[r, b, r] = h_post[c, 32b + r//4, r%4]
        hpc = _f16(h_post[c].reshape(64, 32, 4))        # [b, p, i]
        Dg = np.zeros((128, 64, 128), np.float16)
        Dg[r128, :, r128] = hpc[:, r128 // 4, r128 % 4].T
        m = {
            "x": xc,
            "w": np.ascontiguousarray(W.reshape(128, 64 * 128)),
            "dg": np.ascontiguousarray(Dg.reshape(128, 64 * 128)),
            "ho": _f16(h_out[c]),
            "e4": e4,
        }
        in_maps.append(m)
    return in_maps


def kernel(x, h_res, h_out, h_post):
    nc = build_program(iters=1)
    in_maps = make_in_maps(x, h_res, h_out, h_post)
    res = run_bass_kernel_spmd(nc, in_maps, list(range(NCORES)))
    out = np.stack([res.results[c]["out"].reshape(S, N, D)
                    for c in range(NCORES)])
    return out.astype(np.float32)


# revision 5
# speedup vs baseline: 1.2683x; 1.2683x over previous
"""Trainium2 Bass kernel for nn_ModelNew_17411797418162.

Computation (per (b,s) sample):
  mixed = h_res @ x            # [4,4] @ [4,1024]
  out   = mixed * h_out[None,:] + h_post[:,None] * x

Sharding: pure data parallel over the leading batch dim B=8 -> 1 batch/core.

Per-core design (memory-bound). All device I/O is fp16 (host-side cast;
rel err ~7e-4 vs fp32 reference, well under the 2e-2 gate): x 16MB +
out 16MB + h_out 4MB + weights 2MB ~= 38MB -> ~130us DMA roofline/core
at the measured ~293GB/s.

Engine budget per [128,1024] block (64 blocks/core/pass), all under DMA:
- Rewrite out = h_res @ (x * h_out_bcast) + h_post * x.
- h_out replication onto the 4 stream rows runs on the DVE crossbar
  (stream_shuffle, mask=r//4), NOT the PE: h_out rows are pre-staged
  8-per-quadrant by the host DRAM layout (4 small DMAs per superblock,
  same bytes), and the shuffle is bitcast to fp32 to halve its cycles.
- DVE: shuffle (~0.6us) + y = x*h4 (fp16 2x, ~0.6us) + 3/4 of the adds.
- PE: only the two N=512 fp16 block-diagonal mix matmuls (structural
  minimum; PE measured ~1.08ns/row -> ~71us). The earlier PE-broadcast
  variant put 4-6 matmuls/block on the PE and was PE-bound at 213-302us.
- ACT: PSUM evac to fp16 + p = h_post*x (activation scale) ~1.9us.
- Pool: 1/4 of the adds (SBUF fp16 only; Pool cannot touch PSUM).
"""
import numpy as np

import concourse.bacc as bacc
import concourse.tile as tile
import concourse.mybir as mybir
from concourse.bass_utils import run_bass_kernel_spmd

B, S, N, D = 8, 2048, 4, 1024
NCORES = 8
ROWS = S * N              # 8192 flattened rows per core
NSB = 16                  # super-blocks per core (512 rows each)
SUBS = 4                  # sub-blocks (32 samples x 4 streams) per super-block
F32 = mybir.dt.float32
FP16 = mybir.dt.float16

# stream_shuffle mask: out[32s+r] = in[32s + r//4]
MASK = [r // 4 for r in range(32)]

_cache = {}


def build_program(iters: int = 1, mode: str = "full"):
    """Build the SPMD Bass program (one core's view). Cached per (iters, mode).

    mode: "full" = real kernel; ablations for bottleneck isolation:
      "dma" = loads + store only (wrong output values)
      "pe"  = loads, mix matmuls on raw x, ACT evac, store (no DVE/Pool)
    """
    if (iters, mode) in _cache:
        return _cache[(iters, mode)]

    nc = bacc.Bacc("TRN2", target_bir_lowering=False, debug=False)
    x = nc.dram_tensor("x", [ROWS, D], FP16, kind="ExternalInput")
    w = nc.dram_tensor("w", [128, 64 * 128], FP16, kind="ExternalInput")
    # h_out rows pre-staged for the quadrant shuffle: [sb, s, t, k, d]
    # ho3[sb, s, t, k, :] = h_out[128*sb + 32*k + 8*s + t]
    ho3 = nc.dram_tensor("ho3", [NSB, 4, 8, SUBS, D], FP16,
                         kind="ExternalInput")
    hp = nc.dram_tensor("hp", [128, 64], F32, kind="ExternalInput")
    out = nc.dram_tensor("out", [ROWS, D], FP16, kind="ExternalOutput")

    with tile.TileContext(nc) as tc:
        with (
            tc.tile_pool(name="const", bufs=1) as cpool,
            tc.tile_pool(name="big", bufs=3) as bpool,
            tc.tile_pool(name="mid", bufs=4) as mpool,
            tc.tile_pool(name="psum", bufs=4, space="PSUM") as ppool,
        ):
            w_all = cpool.tile([128, 64 * 128], FP16)
            hp_all = cpool.tile([128, 64], F32)

            def body():
                # weight loads inside the body so the For_i slope reflects
                # the full per-pass HBM traffic (they are per-pass inputs)
                nc.gpsimd.dma_start(w_all[:], w.ap())
                nc.gpsimd.dma_start(hp_all[:], hp.ap())
                for sb in range(NSB):
                    x_t = bpool.tile([128, SUBS * D], FP16, tag="x")
                    src = x.ap()[512 * sb:512 * (sb + 1), :].rearrange(
                        "(k p) d -> p k d", k=SUBS)
                    nc.sync.dma_start(
                        x_t[:].rearrange("p (k d) -> p k d", k=SUBS), src)

                    # 32 live h_out rows per k-group, 8 per quadrant
                    tmp = mpool.tile([128, SUBS * D], FP16, tag="tmp")
                    for s in range(4):
                        nc.gpsimd.dma_start(
                            tmp[32 * s:32 * s + 8, :].rearrange(
                                "t (k d) -> t k d", k=SUBS),
                            ho3.ap()[sb, s])

                    out_sb = bpool.tile([128, SUBS * D], FP16, tag="out_sb")

                    for k in range(SUBS):
                        if mode == "dma":
                            continue
                        blk = SUBS * sb + k
                        xk = x_t[:, D * k:D * (k + 1)]

                        if mode == "full":
                            h4_t = mpool.tile([128, D], FP16, tag="h4")
                            nc.vector.stream_shuffle(
                                h4_t[:].bitcast(F32),
                                tmp[:, D * k:D * (k + 1)].bitcast(F32), MASK)
                            y_t = mpool.tile([128, D], FP16, tag="y")
                            nc.vector.tensor_mul(y_t[:], xk, h4_t[:])
                            rhs = y_t[:]
                        else:
                            rhs = xk

                        ps = ppool.tile([128, D], F32, tag="ps")
                        lhsW = w_all[:, 128 * blk:128 * (blk + 1)]
                        for c in range(2):
                            nc.tensor.matmul(
                                ps[:, 512 * c:512 * (c + 1)],
                                lhsW,
                                rhs[:, 512 * c:512 * (c + 1)],
                                start=True, stop=True)

                        if mode == "full":
                            t_t = mpool.tile([128, D], FP16, tag="t")
                            nc.scalar.copy(t_t[:], ps[:])
                            p_t = mpool.tile([128, D], FP16, tag="p")
                            nc.scalar.activation(
                                p_t[:], xk,
                                mybir.ActivationFunctionType.Copy,
                                scale=hp_all[:, blk:blk + 1])
                            dst = out_sb[:, D * k:D * (k + 1)]
                            if k == 3:
                                nc.gpsimd.tensor_add(dst, t_t[:], p_t[:])
                            else:
                                nc.vector.tensor_add(dst, t_t[:], p_t[:])
                        else:
                            nc.scalar.copy(
                                out_sb[:, D * k:D * (k + 1)], ps[:])

                    src_sb = x_t if mode == "dma" else out_sb
                    dst = out.ap()[512 * sb:512 * (sb + 1), :].rearrange(
                        "(k p) d -> p k d", k=SUBS)
                    nc.scalar.dma_start(
                        dst, src_sb[:].rearrange("p (k d) -> p k d", k=SUBS))

            if iters == 1:
                body()
            else:
                with tc.For_i(0, iters, 1):
                    body()

    nc.compile()
    _cache[(iters, mode)] = nc
    return nc


def _f16(a):
    """fp16 cast with subnormal flush (HW engines may flush; make it exact)."""
    h = np.asarray(a, np.float32).astype(np.float16)
    h[np.abs(h.astype(np.float32)) < 2.0 ** -14] = 0
    return h


def make_in_maps(x, h_res, h_out, h_post):
    """Split full inputs into per-core input maps (host-side, layout only)."""
    x = np.ascontiguousarray(x, dtype=np.float32)
    h_res = np.ascontiguousarray(h_res, dtype=np.float32)
    h_out = np.ascontiguousarray(h_out, dtype=np.float32)
    h_post = np.ascontiguousarray(h_post, dtype=np.float32)

    p32 = np.arange(32)

    in_maps = []
    for c in range(NCORES):
        xc = _f16(x[c].reshape(ROWS, D))
        # Block-diagonal mixing weights, laid out [r, (b, col)] so the DMA is
        # contiguous 16KB per partition: W[4p+j, b, 4p+i] = h_res[c, 32b+p, i, j]
        hr = _f16(h_res[c].reshape(64, 32, 4, 4))       # [b, p, i, j]
        W = np.zeros((128, 64, 128), np.float16)
        for i in range(4):
            for j in range(4):
                W[4 * p32 + j, :, 4 * p32 + i] = hr[:, p32, i, j].T
        # h_out staged for the quadrant shuffle:
        # ho3[sb, s, t, k, :] = h_out[128*sb + 32*k + 8*s + t]
        ho3 = _f16(h_out[c]).reshape(NSB, SUBS, 4, 8, D).transpose(0, 2, 3, 1, 4)
        # per-partition h_post scalars: hp[p, blk] = h_post_flat[128*blk + p]
        hpc = np.ascontiguousarray(_f16(h_post[c]).astype(np.float32)
                                   .reshape(64, 128).T)
        in_maps.append({
            "x": xc,
            "w": np.ascontiguousarray(W.reshape(128, 64 * 128)),
            "ho3": np.ascontiguousarray(ho3),
            "hp": hpc,
        })
    return in_maps


def kernel(x, h_res, h_out, h_post):
    nc = build_program(iters=1)
    in_maps = make_in_maps(x, h_res, h_out, h_post)
    res = run_bass_kernel_spmd(nc, in_maps, list(range(NCORES)))
    out = np.stack([res.results[c]["out"].reshape(S, N, D)
                    for c in range(NCORES)])
    return out.astype(np.float32)


# revision 14
# speedup vs baseline: 1.5095x; 1.1902x over previous
"""Trainium2 Bass kernel for nn_ModelNew_17411797418162.

Computation (per (b,s) sample):
  mixed = h_res @ x            # [4,4] @ [4,1024]
  out   = mixed * h_out[None,:] + h_post[:,None] * x

Sharding: pure data parallel over the leading batch dim B=8 -> 1 batch/core.

Per-core design (memory-bound). All device I/O is fp16 (host-side cast;
rel err ~7e-4 vs fp32 reference, well under the 2e-2 gate): x 16MB +
out 16MB + h_out 4MB + weights 2MB ~= 38MB -> ~130us DMA roofline/core
at the measured ~293GB/s.

Engine budget per [128,1024] block (64 blocks/core/pass), all under DMA:
- Rewrite out = h_res @ (x * h_out_bcast) + h_post * x.
- h_out replication onto the 4 stream rows runs on the DVE crossbar
  (stream_shuffle, mask=4*(r//4)), NOT the PE: h_out rows are pre-staged
  at stride-4 partitions (8 live rows per quadrant) by the host DRAM
  layout — one 32-partition DMA per superblock, same total bytes — and
  the shuffle is bitcast to fp32 to halve its cycles.
- DVE: shuffle (~0.6us) + y = x*h4 (fp16 2x, ~0.6us) + add (~0.6us).
- PE: only the two N=512 fp16 block-diagonal mix matmuls (structural
  minimum; PE measured ~1.08ns/row -> ~71us; ablation "pe" rides +6us
  over the DMA floor). The earlier PE-broadcast variant put 4-6
  matmuls/block on the PE and was PE-bound at 213-302us.
- ACT: PSUM evac to fp16 + p = h_post*x (activation scale) ~1.9us.
- Pool: DMA issue only (measured: Pool tensor_add cost +67us/pass -> no
  Pool compute; it also cannot touch PSUM).
"""
import numpy as np

import concourse.bacc as bacc
import concourse.tile as tile
import concourse.mybir as mybir
from concourse.bass_utils import run_bass_kernel_spmd

B, S, N, D = 8, 2048, 4, 1024
NCORES = 8
ROWS = S * N              # 8192 flattened rows per core
NSB = 16                  # super-blocks per core (512 rows each)
SUBS = 4                  # sub-blocks (32 samples x 4 streams) per super-block
F32 = mybir.dt.float32
FP16 = mybir.dt.float16

# stream_shuffle mask: out[32s+r] = in[32s + 4*(r//4)] — live h_out rows sit
# at stride-4 partitions {4m} (8 per quadrant), staged by one DMA per sb
MASK = [4 * (r // 4) for r in range(32)]

_cache = {}


def build_program(iters: int = 1, mode: str = "full"):
    """Build the SPMD Bass program (one core's view). Cached per (iters, mode).

    mode: "full" = real kernel; ablations for bottleneck isolation:
      "dma"    = loads + store only (wrong output values)
      "pe"     = loads, mix matmuls on raw x, ACT evac, store (no DVE/Pool)
      "nopool" = full but all adds on DVE (no Pool compute)
      "noshuf" = full but y = x*tmp-slice (no stream_shuffle)
    """
    if (iters, mode) in _cache:
        return _cache[(iters, mode)]

    nc = bacc.Bacc("TRN2", target_bir_lowering=False, debug=False)
    x = nc.dram_tensor("x", [ROWS, D], FP16, kind="ExternalInput")
    w = nc.dram_tensor("w", [128, 64 * 128], FP16, kind="ExternalInput")
    # h_out rows pre-staged for the quadrant shuffle:
    # ho3[sb, m, k, :] = h_out[128*sb + 32*k + m], landing at partition 4m
    ho3 = nc.dram_tensor("ho3", [NSB, 32, SUBS, D], FP16,
                         kind="ExternalInput")
    hp = nc.dram_tensor("hp", [128, 64], F32, kind="ExternalInput")
    out = nc.dram_tensor("out", [ROWS, D], FP16, kind="ExternalOutput")

    with tile.TileContext(nc) as tc:
        with (
            tc.tile_pool(name="const", bufs=1) as cpool,
            tc.tile_pool(name="big", bufs=3) as bpool,
            tc.tile_pool(name="mid", bufs=4) as mpool,
            tc.tile_pool(name="psum", bufs=4, space="PSUM") as ppool,
        ):
            w_all = cpool.tile([128, 64 * 128], FP16)
            hp_all = cpool.tile([128, 64], F32)

            def body():
                # weight loads inside the body so the For_i slope reflects
                # the full per-pass HBM traffic (they are per-pass inputs)
                nc.gpsimd.dma_start(w_all[:], w.ap())
                nc.gpsimd.dma_start(hp_all[:], hp.ap())
                for sb in range(NSB):
                    x_t = bpool.tile([128, SUBS * D], FP16, tag="x")
                    src = x.ap()[512 * sb:512 * (sb + 1), :].rearrange(
                        "(k p) d -> p k d", k=SUBS)
                    nc.sync.dma_start(
                        x_t[:].rearrange("p (k d) -> p k d", k=SUBS), src)

                    # 32 live h_out rows per k-group at stride-4 partitions,
                    # one 32-partition DMA per superblock
                    tmp = mpool.tile([128, SUBS * D], FP16, tag="tmp")
                    nc.sync.dma_start(
                        tmp[:].rearrange("(m u) (k d) -> u m k d",
                                         u=4, k=SUBS)[0],
                        ho3.ap()[sb])

                    out_sb = bpool.tile([128, SUBS * D], FP16, tag="out_sb")

                    for k in range(SUBS):
                        if mode == "dma":
                            continue
                        blk = SUBS * sb + k
                        xk = x_t[:, D * k:D * (k + 1)]

                        if mode in ("full", "nopool"):
                            h4_t = mpool.tile([128, D], FP16, tag="h4")
                            nc.vector.stream_shuffle(
                                h4_t[:].bitcast(F32),
                                tmp[:, D * k:D * (k + 1)].bitcast(F32), MASK)
                            y_t = mpool.tile([128, D], FP16, tag="y")
                            nc.vector.tensor_mul(y_t[:], xk, h4_t[:])
                            rhs = y_t[:]
                        elif mode == "noshuf":
                            y_t = mpool.tile([128, D], FP16, tag="y")
                            nc.vector.tensor_mul(
                                y_t[:], xk, tmp[:, D * k:D * (k + 1)])
                            rhs = y_t[:]
                        else:
                            rhs = xk

                        ps = ppool.tile([128, D], F32, tag="ps")
                        lhsW = w_all[:, 128 * blk:128 * (blk + 1)]
                        for c in range(2):
                            nc.tensor.matmul(
                                ps[:, 512 * c:512 * (c + 1)],
                                lhsW,
                                rhs[:, 512 * c:512 * (c + 1)],
                                start=True, stop=True)

                        if mode in ("full", "nopool", "noshuf"):
                            t_t = mpool.tile([128, D], FP16, tag="t")
                            nc.scalar.copy(t_t[:], ps[:])
                            p_t = mpool.tile([128, D], FP16, tag="p")
                            nc.scalar.activation(
                                p_t[:], xk,
                                mybir.ActivationFunctionType.Copy,
                                scale=hp_all[:, blk:blk + 1])
                            dst = out_sb[:, D * k:D * (k + 1)]
                            nc.vector.tensor_add(dst, t_t[:], p_t[:])
                        else:
                            nc.scalar.copy(
                                out_sb[:, D * k:D * (k + 1)], ps[:])

                    src_sb = x_t if mode == "dma" else out_sb
                    dst = out.ap()[512 * sb:512 * (sb + 1), :].rearrange(
                        "(k p) d -> p k d", k=SUBS)
                    nc.scalar.dma_start(
                        dst, src_sb[:].rearrange("p (k d) -> p k d", k=SUBS))

            if iters == 1:
                body()
            else:
                with tc.For_i(0, iters, 1):
                    body()

    nc.compile()
    _cache[(iters, mode)] = nc
    return nc


def _f16(a):
    """fp16 cast with subnormal flush (HW engines may flush; make it exact)."""
    h = np.asarray(a, np.float32).astype(np.float16)
    h[np.abs(h.astype(np.float32)) < 2.0 ** -14] = 0
    return h


def make_in_maps(x, h_res, h_out, h_post):
    """Split full inputs into per-core input maps (host-side, layout only)."""
    x = np.ascontiguousarray(x, dtype=np.float32)
    h_res = np.ascontiguousarray(h_res, dtype=np.float32)
    h_out = np.ascontiguousarray(h_out, dtype=np.float32)
    h_post = np.ascontiguousarray(h_post, dtype=np.float32)

    p32 = np.arange(32)

    in_maps = []
    for c in range(NCORES):
        xc = _f16(x[c].reshape(ROWS, D))
        # Block-diagonal mixing weights, laid out [r, (b, col)] so the DMA is
        # contiguous 16KB per partition: W[4p+j, b, 4p+i] = h_res[c, 32b+p, i, j]
        hr = _f16(h_res[c].reshape(64, 32, 4, 4))       # [b, p, i, j]
        W = np.zeros((128, 64, 128), np.float16)
        for i in range(4):
            for j in range(4):
                W[4 * p32 + j, :, 4 * p32 + i] = hr[:, p32, i, j].T
        # h_out staged for the quadrant shuffle:
        # ho3[sb, m, k, :] = h_out[128*sb + 32*k + m]
        ho3 = _f16(h_out[c]).reshape(NSB, SUBS, 32, D).transpose(0, 2, 1, 3)
        # per-partition h_post scalars: hp[p, blk] = h_post_flat[128*blk + p]
        hpc = np.ascontiguousarray(_f16(h_post[c]).astype(np.float32)
                                   .reshape(64, 128).T)
        in_maps.append({
            "x": xc,
            "w": np.ascontiguousarray(W.reshape(128, 64 * 128)),
            "ho3": np.ascontiguousarray(ho3),
            "hp": hpc,
        })
    return in_maps


def kernel(x, h_res, h_out, h_post):
    nc = build_program(iters=1)
    in_maps = make_in_maps(x, h_res, h_out, h_post)
    res = run_bass_kernel_spmd(nc, in_maps, list(range(NCORES)))
    out = np.stack([res.results[c]["out"].reshape(S, N, D)
                    for c in range(NCORES)])
    return out.astype(np.float32)


# revision 19
# speedup vs baseline: 1.5999x; 1.0599x over previous
"""Trainium2 Bass kernel for nn_ModelNew_17411797418162.

Computation (per (b,s) sample):
  mixed = h_res @ x            # [4,4] @ [4,1024]
  out   = mixed * h_out[None,:] + h_post[:,None] * x

Sharding: pure data parallel over the leading batch dim B=8 -> 1 batch/core.

Per-core design (memory-bound). All device I/O is fp16 (host-side cast;
rel err ~7e-4 vs fp32 reference, well under the 2e-2 gate): x 16MB +
out 16MB + h_out 4MB + weights 2MB ~= 38MB -> ~130us DMA roofline/core
at the measured ~293GB/s.

Engine budget per [128,1024] block (64 blocks/core/pass), all under DMA:
- Rewrite out = h_res @ (x * h_out_bcast) + h_post * x.
- h_out replication onto the 4 stream rows runs on the DVE crossbar
  (stream_shuffle, mask=4*(r//4)), NOT the PE: h_out rows are pre-staged
  at stride-4 partitions (8 live rows per quadrant) by the host DRAM
  layout — one 32-partition DMA per superblock, same total bytes — and
  the shuffle is bitcast to fp32 to halve its cycles.
- DVE: shuffle (~0.6us) + y = x*h4 (fp16 2x, ~0.6us) + add (~0.6us).
- PE: only the two N=512 fp16 block-diagonal mix matmuls (structural
  minimum; PE measured ~1.08ns/row -> ~71us; ablation "pe" rides +6us
  over the DMA floor). The earlier PE-broadcast variant put 4-6
  matmuls/block on the PE and was PE-bound at 213-302us.
- ACT: PSUM evac to fp16 + p = h_post*x (activation scale) ~1.9us.
- Pool: DMA issue only (measured: Pool tensor_add cost +67us/pass -> no
  Pool compute; it also cannot touch PSUM).
"""
import numpy as np

import concourse.bacc as bacc
import concourse.tile as tile
import concourse.mybir as mybir
from concourse.bass_utils import run_bass_kernel_spmd

B, S, N, D = 8, 2048, 4, 1024
NCORES = 8
ROWS = S * N              # 8192 flattened rows per core
NSB = 16                  # super-blocks per core (512 rows each)
SUBS = 4                  # sub-blocks (32 samples x 4 streams) per super-block
F32 = mybir.dt.float32
FP16 = mybir.dt.float16

# stream_shuffle mask: out[32s+r] = in[32s + 4*(r//4)] — live h_out rows sit
# at stride-4 partitions {4m} (8 per quadrant), staged by one DMA per sb
MASK = [4 * (r // 4) for r in range(32)]

_cache = {}


def build_program(iters: int = 1, mode: str = "full"):
    """Build the SPMD Bass program (one core's view). Cached per (iters, mode).

    mode: "full" = real kernel; ablations for bottleneck isolation:
      "dma"    = loads + store only (wrong output values)
      "pe"     = loads, mix matmuls on raw x, ACT evac, store (no DVE/Pool)
      "nopool" = full but all adds on DVE (no Pool compute)
      "noshuf" = full but y = x*tmp-slice (no stream_shuffle)
      "tmpg"   = full but tmp staging DMA on the gpsimd (SWDGE) ring
      "shufpe" = shuffle + y + matmuls + direct evac (no p/add epilogue)
      "deep"   = full with deeper tile rings (mid bufs=6, big bufs=4)
    """
    if (iters, mode) in _cache:
        return _cache[(iters, mode)]

    nc = bacc.Bacc("TRN2", target_bir_lowering=False, debug=False)
    x = nc.dram_tensor("x", [ROWS, D], FP16, kind="ExternalInput")
    w = nc.dram_tensor("w", [128, 64 * 128], FP16, kind="ExternalInput")
    dg = nc.dram_tensor("dg", [128, 64 * 128], FP16, kind="ExternalInput")
    # h_out rows pre-staged for the quadrant shuffle:
    # ho3[sb, m, k, :] = h_out[128*sb + 32*k + m], landing at partition 4m
    ho3 = nc.dram_tensor("ho3", [NSB, 32, SUBS, D], FP16,
                         kind="ExternalInput")
    hp = nc.dram_tensor("hp", [128, 64], F32, kind="ExternalInput")
    out = nc.dram_tensor("out", [ROWS, D], FP16, kind="ExternalOutput")

    with tile.TileContext(nc) as tc:
        with (
            tc.tile_pool(name="const", bufs=1) as cpool,
            tc.tile_pool(name="big", bufs=4 if mode == "deep" else 3) as bpool,
            tc.tile_pool(name="mid", bufs=6 if mode == "deep" else 4) as mpool,
            tc.tile_pool(name="psum", bufs=4, space="PSUM") as ppool,
        ):
            w_all = cpool.tile([128, 64 * 128], FP16)
            d_all = cpool.tile([128, 64 * 128], FP16)
            hp_all = cpool.tile([128, 64], F32)

            def body():
                # weight loads inside the body so the For_i slope reflects
                # the full per-pass HBM traffic (they are per-pass inputs)
                nc.gpsimd.dma_start(w_all[:], w.ap())
                nc.gpsimd.dma_start(d_all[:], dg.ap())
                nc.gpsimd.dma_start(hp_all[:], hp.ap())
                for sb in range(NSB):
                    x_t = bpool.tile([128, SUBS * D], FP16, tag="x")
                    src = x.ap()[512 * sb:512 * (sb + 1), :].rearrange(
                        "(k p) d -> p k d", k=SUBS)
                    nc.sync.dma_start(
                        x_t[:].rearrange("p (k d) -> p k d", k=SUBS), src)

                    # 32 live h_out rows per k-group at stride-4 partitions,
                    # one 32-partition DMA per superblock
                    tmp = mpool.tile([128, SUBS * D], FP16, tag="tmp")
                    eng = nc.gpsimd if mode == "tmpg" else nc.sync
                    eng.dma_start(
                        tmp[:].rearrange("(m u) (k d) -> u m k d",
                                         u=4, k=SUBS)[0],
                        ho3.ap()[sb])

                    out_sb = bpool.tile([128, SUBS * D], FP16, tag="out_sb")

                    for k in range(SUBS):
                        if mode == "dma":
                            continue
                        blk = SUBS * sb + k
                        xk = x_t[:, D * k:D * (k + 1)]

                        if mode in ("full", "nopool", "tmpg", "shufpe", "deep"):
                            h4_t = mpool.tile([128, D], FP16, tag="h4")
                            nc.vector.stream_shuffle(
                                h4_t[:].bitcast(F32),
                                tmp[:, D * k:D * (k + 1)].bitcast(F32), MASK)
                            y_t = mpool.tile([128, D], FP16, tag="y")
                            nc.vector.tensor_mul(y_t[:], xk, h4_t[:])
                            rhs = y_t[:]
                        elif mode == "noshuf":
                            y_t = mpool.tile([128, D], FP16, tag="y")
                            nc.vector.tensor_mul(
                                y_t[:], xk, tmp[:, D * k:D * (k + 1)])
                            rhs = y_t[:]
                        else:
                            rhs = xk

                        ps = ppool.tile([128, D], F32, tag="ps")
                        lhsW = w_all[:, 128 * blk:128 * (blk + 1)]
                        lhsD = d_all[:, 128 * blk:128 * (blk + 1)]
                        epi = mode in ("full", "nopool", "noshuf", "tmpg",
                                       "deep")
                        for c in range(2):
                            one_shot = (c == 1) or not epi
                            nc.tensor.matmul(
                                ps[:, 512 * c:512 * (c + 1)],
                                lhsW,
                                rhs[:, 512 * c:512 * (c + 1)],
                                start=True, stop=one_shot)
                            if not one_shot:
                                # cols 0:512 also get diag(h_post) @ x
                                nc.tensor.matmul(
                                    ps[:, 0:512],
                                    lhsD,
                                    x_t[:, D * k:D * k + 512],
                                    start=False, stop=True)

                        if epi:
                            # cols 0:512 of ps are final (diag accumulated);
                            # cols 512:1024 still need + h_post*x
                            nc.scalar.copy(out_sb[:, D * k:D * (k + 1)],
                                           ps[:])
                            p_t = mpool.tile([128, 512], FP16, tag="p")
                            nc.scalar.activation(
                                p_t[:], x_t[:, D * k + 512:D * (k + 1)],
                                mybir.ActivationFunctionType.Copy,
                                scale=hp_all[:, blk:blk + 1])
                            dst = out_sb[:, D * k + 512:D * (k + 1)]
                            nc.vector.tensor_add(dst, dst, p_t[:])
                        else:
                            nc.scalar.copy(
                                out_sb[:, D * k:D * (k + 1)], ps[:])

                    src_sb = x_t if mode == "dma" else out_sb
                    dst = out.ap()[512 * sb:512 * (sb + 1), :].rearrange(
                        "(k p) d -> p k d", k=SUBS)
                    nc.scalar.dma_start(
                        dst, src_sb[:].rearrange("p (k d) -> p k d", k=SUBS))

            if iters == 1:
                body()
            else:
                with tc.For_i(0, iters, 1):
                    body()

    nc.compile()
    _cache[(iters, mode)] = nc
    return nc


def _f16(a):
    """fp16 cast with subnormal flush (HW engines may flush; make it exact)."""
    h = np.asarray(a, np.float32).astype(np.float16)
    h[np.abs(h.astype(np.float32)) < 2.0 ** -14] = 0
    return h


def make_in_maps(x, h_res, h_out, h_post):
    """Split full inputs into per-core input maps (host-side, layout only)."""
    x = np.ascontiguousarray(x, dtype=np.float32)
    h_res = np.ascontiguousarray(h_res, dtype=np.float32)
    h_out = np.ascontiguousarray(h_out, dtype=np.float32)
    h_post = np.ascontiguousarray(h_post, dtype=np.float32)

    p32 = np.arange(32)

    in_maps = []
    for c in range(NCORES):
        xc = _f16(x[c].reshape(ROWS, D))
        # Block-diagonal mixing weights, laid out [r, (b, col)] so the DMA is
        # contiguous 16KB per partition: W[4p+j, b, 4p+i] = h_res[c, 32b+p, i, j]
        hr = _f16(h_res[c].reshape(64, 32, 4, 4))       # [b, p, i, j]
        W = np.zeros((128, 64, 128), np.float16)
        for i in range(4):
            for j in range(4):
                W[4 * p32 + j, :, 4 * p32 + i] = hr[:, p32, i, j].T
        # Diagonal h_post weights: Dg[r, b, r] = h_post[c, 32b + r//4, r%4]
        r128 = np.arange(128)
        hpc4 = _f16(h_post[c].reshape(64, 32, 4))       # [b, p, i]
        Dg = np.zeros((128, 64, 128), np.float16)
        Dg[r128, :, r128] = hpc4[:, r128 // 4, r128 % 4].T
        # h_out staged for the quadrant shuffle:
        # ho3[sb, m, k, :] = h_out[128*sb + 32*k + m]
        ho3 = _f16(h_out[c]).reshape(NSB, SUBS, 32, D).transpose(0, 2, 1, 3)
        # per-partition h_post scalars: hp[p, blk] = h_post_flat[128*blk + p]
        hpc = np.ascontiguousarray(_f16(h_post[c]).astype(np.float32)
                                   .reshape(64, 128).T)
        in_maps.append({
            "x": xc,
            "w": np.ascontiguousarray(W.reshape(128, 64 * 128)),
            "dg": np.ascontiguousarray(Dg.reshape(128, 64 * 128)),
            "ho3": np.ascontiguousarray(ho3),
            "hp": hpc,
        })
    return in_maps


def kernel(x, h_res, h_out, h_post):
    nc = build_program(iters=1)
    in_maps = make_in_maps(x, h_res, h_out, h_post)
    res = run_bass_kernel_spmd(nc, in_maps, list(range(NCORES)))
    out = np.stack([res.results[c]["out"].reshape(S, N, D)
                    for c in range(NCORES)])
    return out.astype(np.float32)


# revision 20
# speedup vs baseline: 2.0371x; 1.2733x over previous
"""Trainium2 Bass kernel for nn_ModelNew_17411797418162.

Computation (per (b,s) sample):
  mixed = h_res @ x            # [4,4] @ [4,1024]
  out   = mixed * h_out[None,:] + h_post[:,None] * x

Sharding: pure data parallel over the leading batch dim B=8 -> 1 batch/core.

Per-core design (memory-bound). All device I/O is fp16 (host-side cast;
rel err ~7e-4 vs fp32 reference, well under the 2e-2 gate): x 16MB +
out 16MB + h_out 4MB + weights 2MB ~= 38MB -> ~130us DMA roofline/core
at the measured ~293GB/s.

Engine budget per [128,1024] block (64 blocks/core/pass), all under DMA:
- Rewrite out = h_res @ (x * h_out_bcast) + h_post * x.
- h_out replication onto the 4 stream rows runs on the DVE crossbar
  (stream_shuffle, mask=4*(r//4)), NOT the PE: h_out rows are pre-staged
  at stride-4 partitions (8 live rows per quadrant) by the host DRAM
  layout — one 32-partition DMA per superblock, same total bytes — and
  the shuffle is bitcast to fp32 to halve its cycles.
- DVE: shuffle (~0.6us) + y = x*h4 (fp16 2x, ~0.6us) + add (~0.6us).
- PE: only the two N=512 fp16 block-diagonal mix matmuls (structural
  minimum; PE measured ~1.08ns/row -> ~71us; ablation "pe" rides +6us
  over the DMA floor). The earlier PE-broadcast variant put 4-6
  matmuls/block on the PE and was PE-bound at 213-302us.
- ACT: PSUM evac to fp16 + p = h_post*x (activation scale) ~1.9us.
- Pool: DMA issue only (measured: Pool tensor_add cost +67us/pass -> no
  Pool compute; it also cannot touch PSUM).
"""
import numpy as np

import concourse.bacc as bacc
import concourse.tile as tile
import concourse.mybir as mybir
from concourse.bass_utils import run_bass_kernel_spmd

B, S, N, D = 8, 2048, 4, 1024
NCORES = 8
ROWS = S * N              # 8192 flattened rows per core
NSB = 16                  # super-blocks per core (512 rows each)
SUBS = 4                  # sub-blocks (32 samples x 4 streams) per super-block
F32 = mybir.dt.float32
FP16 = mybir.dt.float16

# stream_shuffle mask: out[32s+r] = in[32s + 4*(r//4)] — live h_out rows sit
# at stride-4 partitions {4m} (8 per quadrant), staged by one DMA per sb
MASK = [4 * (r // 4) for r in range(32)]

_cache = {}


def build_program(iters: int = 1, mode: str = "full"):
    """Build the SPMD Bass program (one core's view). Cached per (iters, mode).

    mode: "full" = real kernel; ablations for bottleneck isolation:
      "dma"    = loads + store only (wrong output values)
      "pe"     = loads, mix matmuls on raw x, ACT evac, store (no DVE/Pool)
      "nopool" = full but all adds on DVE (no Pool compute)
      "noshuf" = full but y = x*tmp-slice (no stream_shuffle)
      "tmpg"   = full but tmp staging DMA on the gpsimd (SWDGE) ring
      "shufpe" = shuffle + y + matmuls + direct evac (no p/add epilogue)
      "deep"   = full with deeper tile rings (mid bufs=6, big bufs=4)
    """
    if (iters, mode) in _cache:
        return _cache[(iters, mode)]

    nc = bacc.Bacc("TRN2", target_bir_lowering=False, debug=False)
    x = nc.dram_tensor("x", [ROWS, D], FP16, kind="ExternalInput")
    w = nc.dram_tensor("w", [128, 64 * 128], FP16, kind="ExternalInput")
    dg = nc.dram_tensor("dg", [128, 64 * 128], FP16, kind="ExternalInput")
    # h_out rows pre-staged for the quadrant shuffle:
    # ho3[sb, m, k, :] = h_out[128*sb + 32*k + m], landing at partition 4m
    ho3 = nc.dram_tensor("ho3", [NSB, 32, SUBS, D], FP16,
                         kind="ExternalInput")
    hp = nc.dram_tensor("hp", [128, 64], F32, kind="ExternalInput")
    out = nc.dram_tensor("out", [ROWS, D], FP16, kind="ExternalOutput")

    with tile.TileContext(nc) as tc:
        with (
            tc.tile_pool(name="const", bufs=1) as cpool,
            tc.tile_pool(name="big", bufs=4 if mode == "deep" else 3) as bpool,
            tc.tile_pool(name="mid", bufs=6 if mode == "deep" else 4) as mpool,
            tc.tile_pool(name="psum", bufs=2 if mode == "full" else 4,
                         space="PSUM") as ppool,
        ):
            w_all = cpool.tile([128, 64 * 128], FP16)
            d_all = cpool.tile([128, 64 * 128], FP16)
            hp_all = cpool.tile([128, 64], F32)

            def body():
                # weight loads inside the body so the For_i slope reflects
                # the full per-pass HBM traffic (they are per-pass inputs)
                nc.gpsimd.dma_start(w_all[:], w.ap())
                nc.gpsimd.dma_start(d_all[:], dg.ap())
                nc.gpsimd.dma_start(hp_all[:], hp.ap())
                for sb in range(NSB):
                    x_t = bpool.tile([128, SUBS * D], FP16, tag="x")
                    src = x.ap()[512 * sb:512 * (sb + 1), :].rearrange(
                        "(k p) d -> p k d", k=SUBS)
                    nc.sync.dma_start(
                        x_t[:].rearrange("p (k d) -> p k d", k=SUBS), src)

                    # 32 live h_out rows per k-group at stride-4 partitions,
                    # one 32-partition DMA per superblock
                    tmp = mpool.tile([128, SUBS * D], FP16, tag="tmp")
                    eng = nc.gpsimd if mode == "tmpg" else nc.sync
                    eng.dma_start(
                        tmp[:].rearrange("(m u) (k d) -> u m k d",
                                         u=4, k=SUBS)[0],
                        ho3.ap()[sb])

                    out_sb = bpool.tile([128, SUBS * D], FP16, tag="out_sb")

                    if mode == "full":
                        # paired k-blocks: halves the ACT/DVE instruction
                        # count (ACT has no exec queue; fewer, fatter ops)
                        for j in range(2):
                            base = 2 * D * j
                            h4p = mpool.tile([128, 2 * D], FP16, tag="h4")
                            for t in range(2):
                                k = 2 * j + t
                                nc.vector.stream_shuffle(
                                    h4p[:, D * t:D * (t + 1)].bitcast(F32),
                                    tmp[:, D * k:D * (k + 1)].bitcast(F32),
                                    MASK)
                            y_p = mpool.tile([128, 2 * D], FP16, tag="y")
                            nc.vector.tensor_mul(
                                y_p[:], x_t[:, base:base + 2 * D], h4p[:])
                            ps = ppool.tile([128, 2 * D], F32, tag="ps")
                            for t in range(2):
                                k = 2 * j + t
                                blk = SUBS * sb + k
                                lhsW = w_all[:, 128 * blk:128 * (blk + 1)]
                                lhsD = d_all[:, 128 * blk:128 * (blk + 1)]
                                nc.tensor.matmul(
                                    ps[:, D * t:D * t + 512], lhsW,
                                    y_p[:, D * t:D * t + 512],
                                    start=True, stop=False)
                                nc.tensor.matmul(
                                    ps[:, D * t:D * t + 512], lhsD,
                                    x_t[:, D * k:D * k + 512],
                                    start=False, stop=True)
                                nc.tensor.matmul(
                                    ps[:, D * t + 512:D * (t + 1)], lhsW,
                                    y_p[:, D * t + 512:D * (t + 1)],
                                    start=True, stop=True)
                            nc.scalar.copy(
                                out_sb[:, base:base + 2 * D], ps[:])
                            p_p = mpool.tile([128, D], FP16, tag="p")
                            for t in range(2):
                                k = 2 * j + t
                                blk = SUBS * sb + k
                                nc.vector.tensor_scalar_mul(
                                    p_p[:, 512 * t:512 * (t + 1)],
                                    x_t[:, D * k + 512:D * (k + 1)],
                                    hp_all[:, blk:blk + 1])
                            dst = out_sb[:, base:base + 2 * D].rearrange(
                                "p (t h d) -> p t h d", t=2, h=2)[:, :, 1, :]
                            nc.vector.tensor_add(
                                dst, dst,
                                p_p[:].rearrange("p (t d) -> p t d", t=2))
                        store_src = out_sb
                        dst = out.ap()[512 * sb:512 * (sb + 1), :].rearrange(
                            "(k p) d -> p k d", k=SUBS)
                        nc.scalar.dma_start(
                            dst,
                            store_src[:].rearrange("p (k d) -> p k d", k=SUBS))
                        continue

                    for k in range(SUBS):
                        if mode == "dma":
                            continue
                        blk = SUBS * sb + k
                        xk = x_t[:, D * k:D * (k + 1)]

                        if mode in ("full", "nopool", "tmpg", "shufpe", "deep"):
                            h4_t = mpool.tile([128, D], FP16, tag="h4")
                            nc.vector.stream_shuffle(
                                h4_t[:].bitcast(F32),
                                tmp[:, D * k:D * (k + 1)].bitcast(F32), MASK)
                            y_t = mpool.tile([128, D], FP16, tag="y")
                            nc.vector.tensor_mul(y_t[:], xk, h4_t[:])
                            rhs = y_t[:]
                        elif mode == "noshuf":
                            y_t = mpool.tile([128, D], FP16, tag="y")
                            nc.vector.tensor_mul(
                                y_t[:], xk, tmp[:, D * k:D * (k + 1)])
                            rhs = y_t[:]
                        else:
                            rhs = xk

                        ps = ppool.tile([128, D], F32, tag="ps")
                        lhsW = w_all[:, 128 * blk:128 * (blk + 1)]
                        lhsD = d_all[:, 128 * blk:128 * (blk + 1)]
                        epi = mode in ("full", "nopool", "noshuf", "tmpg",
                                       "deep")
                        for c in range(2):
                            one_shot = (c == 1) or not epi
                            nc.tensor.matmul(
                                ps[:, 512 * c:512 * (c + 1)],
                                lhsW,
                                rhs[:, 512 * c:512 * (c + 1)],
                                start=True, stop=one_shot)
                            if not one_shot:
                                # cols 0:512 also get diag(h_post) @ x
                                nc.tensor.matmul(
                                    ps[:, 0:512],
                                    lhsD,
                                    x_t[:, D * k:D * k + 512],
                                    start=False, stop=True)

                        if epi:
                            # cols 0:512 of ps are final (diag accumulated);
                            # cols 512:1024 still need + h_post*x
                            nc.scalar.copy(out_sb[:, D * k:D * (k + 1)],
                                           ps[:])
                            p_t = mpool.tile([128, 512], FP16, tag="p")
                            nc.scalar.activation(
                                p_t[:], x_t[:, D * k + 512:D * (k + 1)],
                                mybir.ActivationFunctionType.Copy,
                                scale=hp_all[:, blk:blk + 1])
                            dst = out_sb[:, D * k + 512:D * (k + 1)]
                            nc.vector.tensor_add(dst, dst, p_t[:])
                        else:
                            nc.scalar.copy(
                                out_sb[:, D * k:D * (k + 1)], ps[:])

                    src_sb = x_t if mode == "dma" else out_sb
                    dst = out.ap()[512 * sb:512 * (sb + 1), :].rearrange(
                        "(k p) d -> p k d", k=SUBS)
                    nc.scalar.dma_start(
                        dst, src_sb[:].rearrange("p (k d) -> p k d", k=SUBS))

            if iters == 1:
                body()
            else:
                with tc.For_i(0, iters, 1):
                    body()

    nc.compile()
    _cache[(iters, mode)] = nc
    return nc


def _f16(a):
    """fp16 cast with subnormal flush (HW engines may flush; make it exact)."""
    h = np.asarray(a, np.float32).astype(np.float16)
    h[np.abs(h.astype(np.float32)) < 2.0 ** -14] = 0
    return h


def make_in_maps(x, h_res, h_out, h_post):
    """Split full inputs into per-core input maps (host-side, layout only)."""
    x = np.ascontiguousarray(x, dtype=np.float32)
    h_res = np.ascontiguousarray(h_res, dtype=np.float32)
    h_out = np.ascontiguousarray(h_out, dtype=np.float32)
    h_post = np.ascontiguousarray(h_post, dtype=np.float32)

    p32 = np.arange(32)

    in_maps = []
    for c in range(NCORES):
        xc = _f16(x[c].reshape(ROWS, D))
        # Block-diagonal mixing weights, laid out [r, (b, col)] so the DMA is
        # contiguous 16KB per partition: W[4p+j, b, 4p+i] = h_res[c, 32b+p, i, j]
        hr = _f16(h_res[c].reshape(64, 32, 4, 4))       # [b, p, i, j]
        W = np.zeros((128, 64, 128), np.float16)
        for i in range(4):
            for j in range(4):
                W[4 * p32 + j, :, 4 * p32 + i] = hr[:, p32, i, j].T
        # Diagonal h_post weights: Dg[r, b, r] = h_post[c, 32b + r//4, r%4]
        r128 = np.arange(128)
        hpc4 = _f16(h_post[c].reshape(64, 32, 4))       # [b, p, i]
        Dg = np.zeros((128, 64, 128), np.float16)
        Dg[r128, :, r128] = hpc4[:, r128 // 4, r128 % 4].T
        # h_out staged for the quadrant shuffle:
        # ho3[sb, m, k, :] = h_out[128*sb + 32*k + m]
        ho3 = _f16(h_out[c]).reshape(NSB, SUBS, 32, D).transpose(0, 2, 1, 3)
        # per-partition h_post scalars: hp[p, blk] = h_post_flat[128*blk + p]
        hpc = np.ascontiguousarray(_f16(h_post[c]).astype(np.float32)
                                   .reshape(64, 128).T)
        in_maps.append({
            "x": xc,
            "w": np.ascontiguousarray(W.reshape(128, 64 * 128)),
            "dg": np.ascontiguousarray(Dg.reshape(128, 64 * 128)),
            "ho3": np.ascontiguousarray(ho3),
            "hp": hpc,
        })
    return in_maps


def kernel(x, h_res, h_out, h_post):
    nc = build_program(iters=1)
    in_maps = make_in_maps(x, h_res, h_out, h_post)
    res = run_bass_kernel_spmd(nc, in_maps, list(range(NCORES)))
    out = np.stack([res.results[c]["out"].reshape(S, N, D)
                    for c in range(NCORES)])
    return out.astype(np.float32)
